# revision 13
# baseline (speedup 1.0000x reference)
"""Trainium2 Bass kernel for nn_Detect: 3-level detection head.

Math (per level, reference):
    k = conv1x1(x, k_w) + k_b          # [b, 3*512, h, w]
    v = conv1x1(x, v_w) + v_b
    kv = k * v  (flattened to [n, 512] per anchor)
    r[n, t, o] = sum_d kv[n,d] * q[t,d] * out_w[o,d] + out_b[o]
with q = target @ q_w.T + q_b.

Device strategy (8 cores, SPMD), all matmuls fp32r (fast fp32 mode):
  - L0 (x-heavy): shard pixels (h axis) 8 ways; weights replicated;
    full r on device.
  - L1: 2D shard: core = (pixel quarter, otile half) -> 6 otile tasks of
    512 pixels each; weights sharded; r partials summed on host.
  - L2 (weight-heavy): core = (pixel half, otile third) -> 3 otile tasks
    of 256 pixels; weights sharded; r partials summed on host.
  - Partial r bias: out_b/4 per otile partial (4 dchunks -> exact sum).
  - W2T[d, t*5+j] = q[t,d]*out_w[d?,j] computed on device (q GEMM on PE,
    outer products + per-task dchunk one-hot selection on GpSimd, which
    is otherwise idle -- keeps DVE free for PSUM kv evacuation).
  - PE warm-up dummies on x0 bridge the initial weight-DMA wait (HAM).
  - host does layout only: transpose/reshape/shard/unshard/partial-sum.
"""
import sys

import numpy as np

try:
    import concourse.bacc as bacc  # noqa: F401
except Exception:  # pragma: no cover
    sys.path.insert(0, "/opt/trn_rl_repo")

import concourse.bacc as bacc
import concourse.bass as bass  # noqa: F401
import concourse.tile as tile
from concourse import mybir
from concourse.bass_utils import run_bass_kernel_spmd

N_CORES = 8
NA, HD, NO, NT, TD = 3, 512, 5, 16, 512
CH = [256, 512, 1024]
HW = [64, 32, 16]
BS = 2
NTJ = NT * NO                               # 80
NB = 512                                    # matmul free-dim block

# L0: pixel shard
HS0 = HW[0] // N_CORES                      # 8 h rows per core
P0 = BS * HS0 * HW[0]                       # 1024 pixels per core

# L1: (quarter, otile-half): 6 tasks of 512 pixels
P1F = BS * HW[1] * HW[1]                    # 2048 full pixels
L1P = 512
L1T = 6

# L2: (half, otile-third): 3 tasks of 256 pixels
P2F = BS * HW[2] * HW[2]                    # 512 full pixels
L2P = 256
L2T = 3

NSLOT = L1T + L2T                           # 9 W2T task slots

# par column layout (per-core)
PC_KB0, PC_VB0 = 0, 12     # L0 biases by otile (12 each)
PC_KB1, PC_VB1 = 24, 30    # L1 per-task biases (6 each)
PC_KB2, PC_VB2 = 36, 39    # L2 per-task biases (3 each)
PC_QB = 42                 # q_b chunks (4)
PC_OW = 46                 # out_w chunks, m-major (20)
PC_OB = 66                 # out_b tiled [80]
PC_OB4 = 67                # out_b/4 tiled (partials)
PC_SEL = 68                # dchunk-selection one-hots (9 slots x 4)
PC_N = 104

MM_DT = mybir.dt.float32r
WARM_MMS = 24              # PE warm-up dummies during initial DMA wait

_STATE = {}


def _l1_tasks(core):
    q, oh = divmod(core, 2)
    return q, [oh * 6 + i for i in range(6)]


def _l2_tasks(core):
    half, og = divmod(core, 4)
    return half, [og * 3 + i for i in range(3)]


def _build():
    f32 = mybir.dt.float32
    mmd = MM_DT
    act_id = mybir.ActivationFunctionType.Identity
    nc = bacc.Bacc("TRN2", target_bir_lowering=False, debug=False,
                   num_devices=N_CORES)

    x0 = nc.dram_tensor("x0", [128, 2 * P0], mmd, kind="ExternalInput")
    w0 = nc.dram_tensor("w0", [128, 6144], mmd, kind="ExternalInput")
    x1 = nc.dram_tensor("x1", [128, 4 * L1P], mmd, kind="ExternalInput")
    w1 = nc.dram_tensor("w1", [128, 6144], mmd, kind="ExternalInput")
    x2 = nc.dram_tensor("x2", [128, 8 * L2P], mmd, kind="ExternalInput")
    w2 = nc.dram_tensor("w2", [128, 6144], mmd, kind="ExternalInput")
    qw = nc.dram_tensor("qw", [128, 4 * TD], mmd, kind="ExternalInput")
    tgt = nc.dram_tensor("tgt", [128, 4 * NT], mmd, kind="ExternalInput")
    par = nc.dram_tensor("par", [128, PC_N], f32, kind="ExternalInput")
    r0 = nc.dram_tensor("r0", [NTJ, NA * P0], f32, kind="ExternalOutput")
    r1 = nc.dram_tensor("r1", [NTJ, L1T * L1P], f32, kind="ExternalOutput")
    r2 = nc.dram_tensor("r2", [NTJ, L2T * L2P], f32, kind="ExternalOutput")

    with tile.TileContext(nc) as tc:
        with (
            tc.tile_pool(name="const", bufs=1) as cpool,
            tc.tile_pool(name="xp", bufs=3) as xpool,
            tc.tile_pool(name="w0p", bufs=3) as w0pool,
            tc.tile_pool(name="wp", bufs=2) as wpool,
            tc.tile_pool(name="vev", bufs=4) as vpool,
            tc.tile_pool(name="kv", bufs=4) as kvpool,
            tc.tile_pool(name="kvt", bufs=3) as kvtpool,
            tc.tile_pool(name="rout", bufs=4) as rpool,
            tc.tile_pool(name="ps", bufs=6, space="PSUM") as pspool,
            tc.tile_pool(name="psr", bufs=2, space="PSUM") as psrpool,
        ):
            # ---- input DMAs: ONE ring, strict consumption order ----
            x0_sb = xpool.tile([128, 2 * P0], mmd, tag="x", name="x0_sb")
            w0_sb = [w0pool.tile([128, 2048], mmd, tag="w0", name=f"w0_{g}")
                     for g in range(3)]
            par_sb = cpool.tile([128, PC_N], f32, tag="par")
            tgt_sb = cpool.tile([128, 4 * NT], mmd, tag="tgt")
            qw_sb = cpool.tile([128, 4 * TD], mmd, tag="qw")
            x1_sb = xpool.tile([128, 4 * L1P], mmd, tag="x", name="x1_sb")
            w1_sb = wpool.tile([128, 6144], mmd, tag="w", name="w1_sb")
            x2_sb = xpool.tile([128, 8 * L2P], mmd, tag="x", name="x2_sb")
            w2_sb = wpool.tile([128, 6144], mmd, tag="w", name="w2_sb")
            nc.sync.dma_start(par_sb[:], par.ap())
            nc.sync.dma_start(tgt_sb[:], tgt.ap())
            nc.sync.dma_start(x0_sb[:], x0.ap())
            nc.sync.dma_start(w0_sb[0][:], w0.ap()[:, 0:2048])
            nc.sync.dma_start(w0_sb[1][:], w0.ap()[:, 2048:4096])
            nc.sync.dma_start(w0_sb[2][:], w0.ap()[:, 4096:6144])
            nc.sync.dma_start(qw_sb[:], qw.ap())
            nc.sync.dma_start(x1_sb[:], x1.ap())
            nc.sync.dma_start(w1_sb[:], w1.ap())
            nc.sync.dma_start(x2_sb[:], x2.ap())
            nc.sync.dma_start(w2_sb[:], w2.ap())

            qT_sb = cpool.tile([128, 4 * NT], f32, tag="qT")

            def emit_q_gemm():
                # q = target @ q_w.T + q_b, computed as qT [512(hd), 16]
                for m in range(4):
                    psq = psrpool.tile([128, NB], f32, tag="psr")
                    for cc in range(4):
                        nc.tensor.matmul(
                            psq[:, :NT],
                            qw_sb[:, cc * TD + m * 128:
                                  cc * TD + (m + 1) * 128],
                            tgt_sb[:, cc * NT:(cc + 1) * NT],
                            start=(cc == 0), stop=(cc == 3),
                        )
                    nc.scalar.activation(
                        qT_sb[:, m * NT:(m + 1) * NT], psq[:, :NT], act_id,
                        bias=par_sb[:, PC_QB + m:PC_QB + m + 1],
                    )

            def emit_warmups():
                # PE warm-up dummies on tgt (arrives first; warms HAM clock)
                for i in range(WARM_MMS):
                    wps = psrpool.tile([128, NB], f32, tag="psr")
                    nc.tensor.matmul(
                        wps[:64, :64], tgt_sb[:, 0:64],
                        tgt_sb[:, 0:64], start=True, stop=True)
                    if i % 8 == 7:
                        wsc = vpool.tile([128, NB], f32, tag="vev")
                        nc.scalar.activation(wsc[:, 0:1], wps[:, 0:1],
                                             act_id, bias=0.0)

            # ---- W2T formation (emitted into DVE idle slots mid-conv) ----
            w2f_sb = cpool.tile([128, 4 * NTJ], mmd, tag="w2f")
            w2f_4d = w2f_sb[:].rearrange("p (m t j) -> p m t j",
                                         m=4, t=NT, j=NO)

            def emit_w2f():
                for m in range(4):
                    for j in range(NO):
                        nc.vector.tensor_scalar_mul(
                            w2f_4d[:, m, :, j],
                            qT_sb[:, m * NT:(m + 1) * NT],
                            par_sb[:, PC_OW + m * NO + j:
                                   PC_OW + m * NO + j + 1],
                        )
            # per-slot dchunk selection (w2task[s] = sum_m w2f[m]*sel[s,m])
            # is emitted inside the task loops to sit in DVE idle slots
            w2task_sb = cpool.tile([128, NSLOT * NTJ], mmd, tag="w2task")

            def emit_w2task_select(s):
                o = s * NTJ
                nc.vector.tensor_scalar_mul(
                    w2task_sb[:, o:o + NTJ], w2f_sb[:, 0:NTJ],
                    par_sb[:, PC_SEL + s * 4:PC_SEL + s * 4 + 1])
                for m in range(1, 4):
                    nc.vector.scalar_tensor_tensor(
                        w2task_sb[:, o:o + NTJ],
                        w2f_sb[:, m * NTJ:(m + 1) * NTJ],
                        par_sb[:, PC_SEL + s * 4 + m:PC_SEL + s * 4 + m + 1],
                        w2task_sb[:, o:o + NTJ],
                        op0=mybir.AluOpType.mult,
                        op1=mybir.AluOpType.add,
                    )

            # ---- L0: pixel-sharded, full r on device ----
            emit_warmups()
            kv_sb = [kvpool.tile([128, NA * P0], mmd, tag="kv",
                                 name=f"kv_d{d}") for d in range(4)]
            for ot in range(12):            # otile = a*4 + dchunk
                if ot == 4:
                    emit_q_gemm()
                a, dchunk = divmod(ot, 4)
                g, lot = divmod(ot, 4)      # w0 third g, local otile
                for pb in range(2):
                    psk = pspool.tile([128, NB], f32, tag="psc")
                    psv = pspool.tile([128, NB], f32, tag="psc")
                    for cc in range(2):
                        nc.tensor.matmul(
                            psk[:],
                            w0_sb[g][:, lot * 512 + cc * 128:
                                   lot * 512 + (cc + 1) * 128],
                            x0_sb[:, cc * P0 + pb * NB:cc * P0 + pb * NB + NB],
                            start=(cc == 0), stop=(cc == 1),
                        )
                    for cc in range(2):
                        nc.tensor.matmul(
                            psv[:],
                            w0_sb[g][:, lot * 512 + 256 + cc * 128:
                                   lot * 512 + 256 + (cc + 1) * 128],
                            x0_sb[:, cc * P0 + pb * NB:cc * P0 + pb * NB + NB],
                            start=(cc == 0), stop=(cc == 1),
                        )
                    v_sb = vpool.tile([128, NB], f32, tag="vev")
                    nc.scalar.activation(
                        v_sb[:], psv[:], act_id,
                        bias=par_sb[:, PC_VB0 + ot:PC_VB0 + ot + 1],
                    )
                    nc.vector.scalar_tensor_tensor(
                        kv_sb[dchunk][:, a * P0 + pb * NB:
                                      a * P0 + pb * NB + NB],
                        psk[:],
                        par_sb[:, PC_KB0 + ot:PC_KB0 + ot + 1],
                        v_sb[:],
                        op0=mybir.AluOpType.add,
                        op1=mybir.AluOpType.mult,
                    )
                if ot == 5:
                    emit_w2f()
                if ot == 8:
                    for _s in range(NSLOT):
                        emit_w2task_select(_s)
            for nb_i in range(NA * P0 // NB):       # 6 r blocks
                psr = psrpool.tile([NTJ, NB], f32, tag="psr")
                for dchunk in range(4):
                    nc.tensor.matmul(
                        psr[:],
                        w2f_sb[:, dchunk * NTJ:(dchunk + 1) * NTJ],
                        kv_sb[dchunk][:, nb_i * NB:nb_i * NB + NB],
                        start=(dchunk == 0), stop=(dchunk == 3),
                    )
                r_sb = rpool.tile([NTJ, NB], f32, tag="rout")
                nc.scalar.activation(
                    r_sb[:], psr[:], act_id,
                    bias=par_sb[:NTJ, PC_OB:PC_OB + 1],
                )
                nc.gpsimd.dma_start(
                    r0.ap()[:, nb_i * NB:nb_i * NB + NB], r_sb[:])

            # ---- L1 (6 tasks x 512 px, 4 cc), L2 (3 tasks x 256 px, 8 cc):
            #      otile-task partials ----
            for lvl in (1, 2):
                ntask = L1T if lvl == 1 else L2T
                npx = L1P if lvl == 1 else L2P
                ncc = 4 if lvl == 1 else 8
                xsb = x1_sb if lvl == 1 else x2_sb
                wsb = w1_sb if lvl == 1 else w2_sb
                rdram = r1 if lvl == 1 else r2
                pckb = PC_KB1 if lvl == 1 else PC_KB2
                pcvb = PC_VB1 if lvl == 1 else PC_VB2
                slot0 = 0 if lvl == 1 else L1T
                for t in range(ntask):
                    psk = pspool.tile([128, NB], f32, tag="psc")
                    psv = pspool.tile([128, NB], f32, tag="psc")
                    kb = (t * 2 + 0) * ncc * 128
                    vb = (t * 2 + 1) * ncc * 128
                    for cc in range(ncc):
                        nc.tensor.matmul(
                            psk[:, :npx],
                            wsb[:, kb + cc * 128:kb + (cc + 1) * 128],
                            xsb[:, cc * npx:(cc + 1) * npx],
                            start=(cc == 0), stop=(cc == ncc - 1),
                        )
                    for cc in range(ncc):
                        nc.tensor.matmul(
                            psv[:, :npx],
                            wsb[:, vb + cc * 128:vb + (cc + 1) * 128],
                            xsb[:, cc * npx:(cc + 1) * npx],
                            start=(cc == 0), stop=(cc == ncc - 1),
                        )
                    v_sb = vpool.tile([128, NB], f32, tag="vev")
                    nc.scalar.activation(
                        v_sb[:, :npx], psv[:, :npx], act_id,
                        bias=par_sb[:, pcvb + t:pcvb + t + 1],
                    )
                    kv_t = kvtpool.tile([128, NB], mmd, tag="kvt")
                    nc.vector.scalar_tensor_tensor(
                        kv_t[:, :npx], psk[:, :npx],
                        par_sb[:, pckb + t:pckb + t + 1],
                        v_sb[:, :npx],
                        op0=mybir.AluOpType.add,
                        op1=mybir.AluOpType.mult,
                    )
                    s = slot0 + t
                    psr = psrpool.tile([NTJ, NB], f32, tag="psr")
                    nc.tensor.matmul(
                        psr[:, :npx],
                        w2task_sb[:, s * NTJ:(s + 1) * NTJ],
                        kv_t[:, :npx],
                        start=True, stop=True,
                    )
                    r_sb = rpool.tile([NTJ, NB], f32, tag="rout")
                    nc.scalar.activation(
                        r_sb[:, :npx], psr[:, :npx], act_id,
                        bias=par_sb[:NTJ, PC_OB4:PC_OB4 + 1],
                    )
                    nc.gpsimd.dma_start(
                        rdram.ap()[:, t * npx:(t + 1) * npx], r_sb[:, :npx])

    nc.compile()
    return nc


def _chunk128(arr):
    """[C, F] -> [128, (C//128)*F] with chunk-major columns."""
    c, f = arr.shape
    return np.ascontiguousarray(
        arr.reshape(c // 128, 128, f).transpose(1, 0, 2).reshape(128, -1))


def _prep(inputs):
    """Host-side layout prep. Returns per-core input maps."""
    mm_np = mybir.dt.np(MM_DT)
    f = lambda k: np.asarray(inputs[k], dtype=np.float32)

    shared = {}
    # L0 weights, replicated: col = g*2048 + lot*512 + kv*256 + cc*128
    kwT0, vwT0 = f("k_w0").T, f("v_w0").T     # [256, 1536]
    w0cols = np.empty((128, 6144), np.float32)
    for ot in range(12):
        g, lot = divmod(ot, 4)
        base = g * 2048 + lot * 512
        w0cols[:, base:base + 256] = _chunk128(kwT0[:, ot * 128:(ot + 1) * 128])
        w0cols[:, base + 256:base + 512] = \
            _chunk128(vwT0[:, ot * 128:(ot + 1) * 128])
    shared["w0"] = w0cols.astype(mm_np)
    shared["qw"] = _chunk128(f("q_w").T).astype(mm_np)
    shared["tgt"] = _chunk128(f("target").T).astype(mm_np)

    kwT1, vwT1 = f("k_w1").T, f("v_w1").T     # [512, 1536]
    kwT2, vwT2 = f("k_w2").T, f("v_w2").T     # [1024, 1536]
    kb1 = f("k_b1").reshape(12, 128)
    vb1 = f("v_b1").reshape(12, 128)
    kb2 = f("k_b2").reshape(12, 128)
    vb2 = f("v_b2").reshape(12, 128)
    ow128 = _chunk128(f("out_w").T)           # [128, 20] m-major

    par_base = np.zeros((128, PC_N), np.float32)
    par_base[:, PC_KB0:PC_KB0 + 12] = f("k_b0").reshape(12, 128).T
    par_base[:, PC_VB0:PC_VB0 + 12] = f("v_b0").reshape(12, 128).T
    par_base[:, PC_QB:PC_QB + 4] = f("q_b").reshape(4, 128).T
    par_base[:, PC_OW:PC_OW + 20] = ow128
    par_base[:NTJ, PC_OB] = np.tile(f("out_b"), NT)
    par_base[:NTJ, PC_OB4] = np.tile(f("out_b"), NT) / 4.0

    xt1 = np.ascontiguousarray(
        f("x1").transpose(1, 0, 2, 3)).reshape(CH[1], -1)   # [512, 2048]
    xt2 = np.ascontiguousarray(
        f("x2").transpose(1, 0, 2, 3)).reshape(CH[2], -1)   # [1024, 512]

    in_maps = []
    for core in range(N_CORES):
        m = dict(shared)
        x = f("x0")[:, :, core * HS0:(core + 1) * HS0, :]
        xt = np.ascontiguousarray(x.transpose(1, 0, 2, 3)).reshape(CH[0], -1)
        m["x0"] = _chunk128(xt).astype(mm_np)

        par = par_base.copy()

        q1, ots1 = _l1_tasks(core)
        m["x1"] = _chunk128(xt1[:, q1 * L1P:(q1 + 1) * L1P]).astype(mm_np)
        w1cols = np.empty((128, 6144), np.float32)
        for t, ot in enumerate(ots1):
            dchunk = ot % 4
            par[:, PC_KB1 + t] = kb1[ot]
            par[:, PC_VB1 + t] = vb1[ot]
            par[:, PC_SEL + t * 4 + dchunk] = 1.0
            w1cols[:, (t * 2) * 512:(t * 2 + 1) * 512] = \
                _chunk128(kwT1[:, ot * 128:(ot + 1) * 128])
            w1cols[:, (t * 2 + 1) * 512:(t * 2 + 2) * 512] = \
                _chunk128(vwT1[:, ot * 128:(ot + 1) * 128])
        m["w1"] = w1cols.astype(mm_np)

        h2, ots2 = _l2_tasks(core)
        m["x2"] = _chunk128(xt2[:, h2 * L2P:(h2 + 1) * L2P]).astype(mm_np)
        w2cols = np.empty((128, 6144), np.float32)
        for t, ot in enumerate(ots2):
            s = L1T + t
            dchunk = ot % 4
            par[:, PC_KB2 + t] = kb2[ot]
            par[:, PC_VB2 + t] = vb2[ot]
            par[:, PC_SEL + s * 4 + dchunk] = 1.0
            w2cols[:, (t * 2) * 1024:(t * 2 + 1) * 1024] = \
                _chunk128(kwT2[:, ot * 128:(ot + 1) * 128])
            w2cols[:, (t * 2 + 1) * 1024:(t * 2 + 2) * 1024] = \
                _chunk128(vwT2[:, ot * 128:(ot + 1) * 128])
        m["w2"] = w2cols.astype(mm_np)

        m["par"] = par
        in_maps.append(m)
    return in_maps


def _assemble(results):
    """Per-core outputs -> tuple of 3 full [2,16,3,h,w,5] arrays."""
    parts = []
    for core in range(N_CORES):
        r = results[core]["r0"].reshape(NT, NO, NA, BS, HS0, HW[0])
        parts.append(r.transpose(3, 0, 2, 4, 5, 1))
    o0 = np.ascontiguousarray(np.concatenate(parts, axis=3))

    full1 = np.zeros((NTJ, NA, P1F), np.float64)
    for core in range(N_CORES):
        q1, ots1 = _l1_tasks(core)
        rc = results[core]["r1"]
        for t, ot in enumerate(ots1):
            full1[:, ot // 4, q1 * L1P:(q1 + 1) * L1P] += \
                rc[:, t * L1P:(t + 1) * L1P]
    o1 = full1.astype(np.float32).reshape(NT, NO, NA, BS, HW[1], HW[1])
    o1 = np.ascontiguousarray(o1.transpose(3, 0, 2, 4, 5, 1))

    full2 = np.zeros((NTJ, NA, P2F), np.float64)
    for core in range(N_CORES):
        h2, ots2 = _l2_tasks(core)
        rc = results[core]["r2"]
        for t, ot in enumerate(ots2):
            full2[:, ot // 4, h2 * L2P:(h2 + 1) * L2P] += \
                rc[:, t * L2P:(t + 1) * L2P]
    o2 = full2.astype(np.float32).reshape(NT, NO, NA, BS, HW[2], HW[2])
    o2 = np.ascontiguousarray(o2.transpose(3, 0, 2, 4, 5, 1))
    return (o0, o1, o2)


def _get_nc():
    if "nc" not in _STATE:
        _STATE["nc"] = _build()
    return _STATE["nc"]


def _run(inputs, **kw):
    nc = _get_nc()
    in_maps = _prep(inputs)
    res = run_bass_kernel_spmd(nc, in_maps, list(range(N_CORES)), **kw)
    return res


def kernel(**inputs):
    res = _run(inputs)
    return _assemble(res.results)


# revision 15
# speedup vs baseline: 1.0041x; 1.0041x over previous
"""Trainium2 Bass kernel for nn_Detect: 3-level detection head.

Math (per level, reference):
    k = conv1x1(x, k_w) + k_b          # [b, 3*512, h, w]
    v = conv1x1(x, v_w) + v_b
    kv = k * v  (flattened to [n, 512] per anchor)
    r[n, t, o] = sum_d kv[n,d] * q[t,d] * out_w[o,d] + out_b[o]
with q = target @ q_w.T + q_b.

Device strategy (8 cores, SPMD), all matmuls fp32r (fast fp32 mode):
  - L0 (x-heavy): shard pixels (h axis) 8 ways; weights replicated;
    full r on device.
  - L1: 2D shard: core = (pixel quarter, otile half) -> 6 otile tasks of
    512 pixels each; weights sharded; r partials summed on host.
  - L2 (weight-heavy): core = (pixel half, otile third) -> 3 otile tasks
    of 256 pixels; weights sharded; r partials summed on host.
  - Partial r bias: out_b/4 per otile partial (4 dchunks -> exact sum).
  - W2T[d, t*5+j] = q[t,d]*out_w[d?,j] computed on device (q GEMM on PE,
    outer products + per-task dchunk one-hot selection on GpSimd, which
    is otherwise idle -- keeps DVE free for PSUM kv evacuation).
  - PE warm-up dummies on x0 bridge the initial weight-DMA wait (HAM).
  - host does layout only: transpose/reshape/shard/unshard/partial-sum.
"""
import sys

import numpy as np

try:
    import concourse.bacc as bacc  # noqa: F401
except Exception:  # pragma: no cover
    sys.path.insert(0, "/opt/trn_rl_repo")

import concourse.bacc as bacc
import concourse.bass as bass  # noqa: F401
import concourse.tile as tile
from concourse import mybir
from concourse.bass_utils import run_bass_kernel_spmd

N_CORES = 8
NA, HD, NO, NT, TD = 3, 512, 5, 16, 512
CH = [256, 512, 1024]
HW = [64, 32, 16]
BS = 2
NTJ = NT * NO                               # 80
NB = 512                                    # matmul free-dim block

# L0: pixel shard
HS0 = HW[0] // N_CORES                      # 8 h rows per core
P0 = BS * HS0 * HW[0]                       # 1024 pixels per core

# L1: (quarter, otile-half): 6 tasks of 512 pixels
P1F = BS * HW[1] * HW[1]                    # 2048 full pixels
L1P = 512
L1T = 6

# L2: (half, otile-third): 3 tasks of 256 pixels
P2F = BS * HW[2] * HW[2]                    # 512 full pixels
L2P = 256
L2T = 3

NSLOT = L1T + L2T                           # 9 W2T task slots

# par column layout (per-core)
PC_KB0, PC_VB0 = 0, 12     # L0 biases by otile (12 each)
PC_KB1, PC_VB1 = 24, 30    # L1 per-task biases (6 each)
PC_KB2, PC_VB2 = 36, 39    # L2 per-task biases (3 each)
PC_QB = 42                 # q_b chunks (4)
PC_OW = 46                 # out_w chunks, m-major (20)
PC_OB = 66                 # out_b tiled [80]
PC_OB4 = 67                # out_b/4 tiled (partials)
PC_SEL = 68                # dchunk-selection one-hots (9 slots x 4)
PC_N = 104

MM_DT = mybir.dt.float32r
WARM_MMS = 10              # PE warm-up dummies during initial DMA wait

_STATE = {}


def _l1_tasks(core):
    q, oh = divmod(core, 2)
    return q, [oh * 6 + i for i in range(6)]


def _l2_tasks(core):
    half, og = divmod(core, 4)
    return half, [og * 3 + i for i in range(3)]


def _build():
    f32 = mybir.dt.float32
    mmd = MM_DT
    act_id = mybir.ActivationFunctionType.Identity
    nc = bacc.Bacc("TRN2", target_bir_lowering=False, debug=False,
                   num_devices=N_CORES)

    x0 = nc.dram_tensor("x0", [128, 2 * P0], mmd, kind="ExternalInput")
    w0 = nc.dram_tensor("w0", [128, 6144], mmd, kind="ExternalInput")
    x1 = nc.dram_tensor("x1", [128, 4 * L1P], mmd, kind="ExternalInput")
    w1 = nc.dram_tensor("w1", [128, 6144], mmd, kind="ExternalInput")
    x2 = nc.dram_tensor("x2", [128, 8 * L2P], mmd, kind="ExternalInput")
    w2 = nc.dram_tensor("w2", [128, 6144], mmd, kind="ExternalInput")
    qw = nc.dram_tensor("qw", [128, 4 * TD], mmd, kind="ExternalInput")
    tgt = nc.dram_tensor("tgt", [128, 4 * NT], mmd, kind="ExternalInput")
    par = nc.dram_tensor("par", [128, PC_N], f32, kind="ExternalInput")
    r0 = nc.dram_tensor("r0", [NTJ, NA * P0], f32, kind="ExternalOutput")
    r1 = nc.dram_tensor("r1", [NTJ, L1T * L1P], f32, kind="ExternalOutput")
    r2 = nc.dram_tensor("r2", [NTJ, L2T * L2P], f32, kind="ExternalOutput")

    with tile.TileContext(nc) as tc:
        with (
            tc.tile_pool(name="const", bufs=1) as cpool,
            tc.tile_pool(name="xp", bufs=3) as xpool,
            tc.tile_pool(name="w0p", bufs=3) as w0pool,
            tc.tile_pool(name="wp", bufs=2) as wpool,
            tc.tile_pool(name="vev", bufs=4) as vpool,
            tc.tile_pool(name="kv", bufs=4) as kvpool,
            tc.tile_pool(name="kvt", bufs=3) as kvtpool,
            tc.tile_pool(name="rout", bufs=4) as rpool,
            tc.tile_pool(name="ps", bufs=6, space="PSUM") as pspool,
            tc.tile_pool(name="psr", bufs=2, space="PSUM") as psrpool,
        ):
            # ---- input DMAs: ONE ring, strict consumption order ----
            x0_sb = xpool.tile([128, 2 * P0], mmd, tag="x", name="x0_sb")
            w0_sb = [w0pool.tile([128, 2048], mmd, tag="w0", name=f"w0_{g}")
                     for g in range(3)]
            par_sb = cpool.tile([128, PC_N], f32, tag="par")
            tgt_sb = cpool.tile([128, 4 * NT], mmd, tag="tgt")
            qw_sb = cpool.tile([128, 4 * TD], mmd, tag="qw")
            x1_sb = xpool.tile([128, 4 * L1P], mmd, tag="x", name="x1_sb")
            w1_sb = wpool.tile([128, 6144], mmd, tag="w", name="w1_sb")
            x2_sb = xpool.tile([128, 8 * L2P], mmd, tag="x", name="x2_sb")
            w2_sb = wpool.tile([128, 6144], mmd, tag="w", name="w2_sb")
            nc.sync.dma_start(par_sb[:], par.ap())
            nc.sync.dma_start(tgt_sb[:], tgt.ap())
            nc.sync.dma_start(x0_sb[:], x0.ap())
            nc.sync.dma_start(w0_sb[0][:], w0.ap()[:, 0:2048])
            nc.sync.dma_start(w0_sb[1][:], w0.ap()[:, 2048:4096])
            nc.sync.dma_start(w0_sb[2][:], w0.ap()[:, 4096:6144])
            nc.sync.dma_start(qw_sb[:], qw.ap())
            nc.sync.dma_start(x1_sb[:], x1.ap())
            nc.sync.dma_start(w1_sb[:], w1.ap())
            nc.sync.dma_start(x2_sb[:], x2.ap())
            nc.sync.dma_start(w2_sb[:], w2.ap())

            qT_sb = cpool.tile([128, 4 * NT], f32, tag="qT")

            def emit_q_gemm():
                # q = target @ q_w.T + q_b, computed as qT [512(hd), 16]
                for m in range(4):
                    psq = psrpool.tile([128, NB], f32, tag="psr")
                    for cc in range(4):
                        nc.tensor.matmul(
                            psq[:, :NT],
                            qw_sb[:, cc * TD + m * 128:
                                  cc * TD + (m + 1) * 128],
                            tgt_sb[:, cc * NT:(cc + 1) * NT],
                            start=(cc == 0), stop=(cc == 3),
                        )
                    nc.scalar.activation(
                        qT_sb[:, m * NT:(m + 1) * NT], psq[:, :NT], act_id,
                        bias=par_sb[:, PC_QB + m:PC_QB + m + 1],
                    )

            def emit_warmups():
                # PE warm-up dummies on tgt (arrives first; warms HAM clock)
                for i in range(WARM_MMS):
                    wps = psrpool.tile([128, NB], f32, tag="psr")
                    nc.tensor.matmul(
                        wps[:64, :64], tgt_sb[:, 0:64],
                        tgt_sb[:, 0:64], start=True, stop=True)
                    if i % 8 == 7:
                        wsc = vpool.tile([128, NB], f32, tag="vev")
                        nc.scalar.activation(wsc[:, 0:1], wps[:, 0:1],
                                             act_id, bias=0.0)

            # ---- W2T formation (emitted into DVE idle slots mid-conv) ----
            w2f_sb = cpool.tile([128, 4 * NTJ], mmd, tag="w2f")
            w2f_4d = w2f_sb[:].rearrange("p (m t j) -> p m t j",
                                         m=4, t=NT, j=NO)

            def emit_w2f():
                for m in range(4):
                    for j in range(NO):
                        nc.vector.tensor_scalar_mul(
                            w2f_4d[:, m, :, j],
                            qT_sb[:, m * NT:(m + 1) * NT],
                            par_sb[:, PC_OW + m * NO + j:
                                   PC_OW + m * NO + j + 1],
                        )
            # per-slot dchunk selection (w2task[s] = sum_m w2f[m]*sel[s,m])
            # is emitted inside the task loops to sit in DVE idle slots
            w2task_sb = cpool.tile([128, NSLOT * NTJ], mmd, tag="w2task")

            def emit_w2task_select(s):
                o = s * NTJ
                nc.vector.tensor_scalar_mul(
                    w2task_sb[:, o:o + NTJ], w2f_sb[:, 0:NTJ],
                    par_sb[:, PC_SEL + s * 4:PC_SEL + s * 4 + 1])
                for m in range(1, 4):
                    nc.vector.scalar_tensor_tensor(
                        w2task_sb[:, o:o + NTJ],
                        w2f_sb[:, m * NTJ:(m + 1) * NTJ],
                        par_sb[:, PC_SEL + s * 4 + m:PC_SEL + s * 4 + m + 1],
                        w2task_sb[:, o:o + NTJ],
                        op0=mybir.AluOpType.mult,
                        op1=mybir.AluOpType.add,
                    )

            # ---- L0: pixel-sharded, full r on device ----
            emit_warmups()
            kv_sb = [kvpool.tile([128, NA * P0], mmd, tag="kv",
                                 name=f"kv_d{d}") for d in range(4)]
            for ot in range(12):            # otile = a*4 + dchunk
                if ot == 4:
                    emit_q_gemm()
                a, dchunk = divmod(ot, 4)
                g, lot = divmod(ot, 4)      # w0 third g, local otile
                for pb in range(2):
                    psk = pspool.tile([128, NB], f32, tag="psc")
                    psv = pspool.tile([128, NB], f32, tag="psc")
                    for cc in range(2):
                        nc.tensor.matmul(
                            psk[:],
                            w0_sb[g][:, lot * 512 + cc * 128:
                                   lot * 512 + (cc + 1) * 128],
                            x0_sb[:, cc * P0 + pb * NB:cc * P0 + pb * NB + NB],
                            start=(cc == 0), stop=(cc == 1),
                        )
                    for cc in range(2):
                        nc.tensor.matmul(
                            psv[:],
                            w0_sb[g][:, lot * 512 + 256 + cc * 128:
                                   lot * 512 + 256 + (cc + 1) * 128],
                            x0_sb[:, cc * P0 + pb * NB:cc * P0 + pb * NB + NB],
                            start=(cc == 0), stop=(cc == 1),
                        )
                    v_sb = vpool.tile([128, NB], f32, tag="vev")
                    nc.scalar.activation(
                        v_sb[:], psv[:], act_id,
                        bias=par_sb[:, PC_VB0 + ot:PC_VB0 + ot + 1],
                    )
                    nc.vector.scalar_tensor_tensor(
                        kv_sb[dchunk][:, a * P0 + pb * NB:
                                      a * P0 + pb * NB + NB],
                        psk[:],
                        par_sb[:, PC_KB0 + ot:PC_KB0 + ot + 1],
                        v_sb[:],
                        op0=mybir.AluOpType.add,
                        op1=mybir.AluOpType.mult,
                    )
                if ot == 5:
                    emit_w2f()
                if ot == 8:
                    for _s in range(NSLOT):
                        emit_w2task_select(_s)
            for nb_i in range(NA * P0 // NB):       # 6 r blocks
                psr = psrpool.tile([NTJ, NB], f32, tag="psr")
                for dchunk in range(4):
                    nc.tensor.matmul(
                        psr[:],
                        w2f_sb[:, dchunk * NTJ:(dchunk + 1) * NTJ],
                        kv_sb[dchunk][:, nb_i * NB:nb_i * NB + NB],
                        start=(dchunk == 0), stop=(dchunk == 3),
                    )
                r_sb = rpool.tile([NTJ, NB], f32, tag="rout")
                nc.scalar.activation(
                    r_sb[:], psr[:], act_id,
                    bias=par_sb[:NTJ, PC_OB:PC_OB + 1],
                )
                nc.gpsimd.dma_start(
                    r0.ap()[:, nb_i * NB:nb_i * NB + NB], r_sb[:])

            # ---- L1 (6 tasks x 512 px, 4 cc), L2 (3 tasks x 256 px, 8 cc):
            #      otile-task partials ----
            for lvl in (1, 2):
                ntask = L1T if lvl == 1 else L2T
                npx = L1P if lvl == 1 else L2P
                ncc = 4 if lvl == 1 else 8
                xsb = x1_sb if lvl == 1 else x2_sb
                wsb = w1_sb if lvl == 1 else w2_sb
                rdram = r1 if lvl == 1 else r2
                pckb = PC_KB1 if lvl == 1 else PC_KB2
                pcvb = PC_VB1 if lvl == 1 else PC_VB2
                slot0 = 0 if lvl == 1 else L1T
                for t in range(ntask):
                    psk = pspool.tile([128, NB], f32, tag="psc")
                    psv = pspool.tile([128, NB], f32, tag="psc")
                    kb = (t * 2 + 0) * ncc * 128
                    vb = (t * 2 + 1) * ncc * 128
                    for cc in range(ncc):
                        nc.tensor.matmul(
                            psk[:, :npx],
                            wsb[:, kb + cc * 128:kb + (cc + 1) * 128],
                            xsb[:, cc * npx:(cc + 1) * npx],
                            start=(cc == 0), stop=(cc == ncc - 1),
                        )
                    for cc in range(ncc):
                        nc.tensor.matmul(
                            psv[:, :npx],
                            wsb[:, vb + cc * 128:vb + (cc + 1) * 128],
                            xsb[:, cc * npx:(cc + 1) * npx],
                            start=(cc == 0), stop=(cc == ncc - 1),
                        )
                    v_sb = vpool.tile([128, NB], f32, tag="vev")
                    nc.scalar.activation(
                        v_sb[:, :npx], psv[:, :npx], act_id,
                        bias=par_sb[:, pcvb + t:pcvb + t + 1],
                    )
                    kv_t = kvtpool.tile([128, NB], mmd, tag="kvt")
                    nc.vector.scalar_tensor_tensor(
                        kv_t[:, :npx], psk[:, :npx],
                        par_sb[:, pckb + t:pckb + t + 1],
                        v_sb[:, :npx],
                        op0=mybir.AluOpType.add,
                        op1=mybir.AluOpType.mult,
                    )
                    s = slot0 + t
                    psr = psrpool.tile([NTJ, NB], f32, tag="psr")
                    nc.tensor.matmul(
                        psr[:, :npx],
                        w2task_sb[:, s * NTJ:(s + 1) * NTJ],
                        kv_t[:, :npx],
                        start=True, stop=True,
                    )
                    r_sb = rpool.tile([NTJ, NB], f32, tag="rout")
                    nc.scalar.activation(
                        r_sb[:, :npx], psr[:, :npx], act_id,
                        bias=par_sb[:NTJ, PC_OB4:PC_OB4 + 1],
                    )
                    nc.gpsimd.dma_start(
                        rdram.ap()[:, t * npx:(t + 1) * npx], r_sb[:, :npx])

    nc.compile()
    return nc


def _chunk128(arr):
    """[C, F] -> [128, (C//128)*F] with chunk-major columns."""
    c, f = arr.shape
    return np.ascontiguousarray(
        arr.reshape(c // 128, 128, f).transpose(1, 0, 2).reshape(128, -1))


def _prep(inputs):
    """Host-side layout prep. Returns per-core input maps."""
    mm_np = mybir.dt.np(MM_DT)
    f = lambda k: np.asarray(inputs[k], dtype=np.float32)

    shared = {}
    # L0 weights, replicated: col = g*2048 + lot*512 + kv*256 + cc*128
    kwT0, vwT0 = f("k_w0").T, f("v_w0").T     # [256, 1536]
    w0cols = np.empty((128, 6144), np.float32)
    for ot in range(12):
        g, lot = divmod(ot, 4)
        base = g * 2048 + lot * 512
        w0cols[:, base:base + 256] = _chunk128(kwT0[:, ot * 128:(ot + 1) * 128])
        w0cols[:, base + 256:base + 512] = \
            _chunk128(vwT0[:, ot * 128:(ot + 1) * 128])
    shared["w0"] = w0cols.astype(mm_np)
    shared["qw"] = _chunk128(f("q_w").T).astype(mm_np)
    shared["tgt"] = _chunk128(f("target").T).astype(mm_np)

    kwT1, vwT1 = f("k_w1").T, f("v_w1").T     # [512, 1536]
    kwT2, vwT2 = f("k_w2").T, f("v_w2").T     # [1024, 1536]
    kb1 = f("k_b1").reshape(12, 128)
    vb1 = f("v_b1").reshape(12, 128)
    kb2 = f("k_b2").reshape(12, 128)
    vb2 = f("v_b2").reshape(12, 128)
    ow128 = _chunk128(f("out_w").T)           # [128, 20] m-major

    par_base = np.zeros((128, PC_N), np.float32)
    par_base[:, PC_KB0:PC_KB0 + 12] = f("k_b0").reshape(12, 128).T
    par_base[:, PC_VB0:PC_VB0 + 12] = f("v_b0").reshape(12, 128).T
    par_base[:, PC_QB:PC_QB + 4] = f("q_b").reshape(4, 128).T
    par_base[:, PC_OW:PC_OW + 20] = ow128
    par_base[:NTJ, PC_OB] = np.tile(f("out_b"), NT)
    par_base[:NTJ, PC_OB4] = np.tile(f("out_b"), NT) / 4.0

    xt1 = np.ascontiguousarray(
        f("x1").transpose(1, 0, 2, 3)).reshape(CH[1], -1)   # [512, 2048]
    xt2 = np.ascontiguousarray(
        f("x2").transpose(1, 0, 2, 3)).reshape(CH[2], -1)   # [1024, 512]

    in_maps = []
    for core in range(N_CORES):
        m = dict(shared)
        x = f("x0")[:, :, core * HS0:(core + 1) * HS0, :]
        xt = np.ascontiguousarray(x.transpose(1, 0, 2, 3)).reshape(CH[0], -1)
        m["x0"] = _chunk128(xt).astype(mm_np)

        par = par_base.copy()

        q1, ots1 = _l1_tasks(core)
        m["x1"] = _chunk128(xt1[:, q1 * L1P:(q1 + 1) * L1P]).astype(mm_np)
        w1cols = np.empty((128, 6144), np.float32)
        for t, ot in enumerate(ots1):
            dchunk = ot % 4
            par[:, PC_KB1 + t] = kb1[ot]
            par[:, PC_VB1 + t] = vb1[ot]
            par[:, PC_SEL + t * 4 + dchunk] = 1.0
            w1cols[:, (t * 2) * 512:(t * 2 + 1) * 512] = \
                _chunk128(kwT1[:, ot * 128:(ot + 1) * 128])
            w1cols[:, (t * 2 + 1) * 512:(t * 2 + 2) * 512] = \
                _chunk128(vwT1[:, ot * 128:(ot + 1) * 128])
        m["w1"] = w1cols.astype(mm_np)

        h2, ots2 = _l2_tasks(core)
        m["x2"] = _chunk128(xt2[:, h2 * L2P:(h2 + 1) * L2P]).astype(mm_np)
        w2cols = np.empty((128, 6144), np.float32)
        for t, ot in enumerate(ots2):
            s = L1T + t
            dchunk = ot % 4
            par[:, PC_KB2 + t] = kb2[ot]
            par[:, PC_VB2 + t] = vb2[ot]
            par[:, PC_SEL + s * 4 + dchunk] = 1.0
            w2cols[:, (t * 2) * 1024:(t * 2 + 1) * 1024] = \
                _chunk128(kwT2[:, ot * 128:(ot + 1) * 128])
            w2cols[:, (t * 2 + 1) * 1024:(t * 2 + 2) * 1024] = \
                _chunk128(vwT2[:, ot * 128:(ot + 1) * 128])
        m["w2"] = w2cols.astype(mm_np)

        m["par"] = par
        in_maps.append(m)
    return in_maps


def _assemble(results):
    """Per-core outputs -> tuple of 3 full [2,16,3,h,w,5] arrays."""
    parts = []
    for core in range(N_CORES):
        r = results[core]["r0"].reshape(NT, NO, NA, BS, HS0, HW[0])
        parts.append(r.transpose(3, 0, 2, 4, 5, 1))
    o0 = np.ascontiguousarray(np.concatenate(parts, axis=3))

    full1 = np.zeros((NTJ, NA, P1F), np.float64)
    for core in range(N_CORES):
        q1, ots1 = _l1_tasks(core)
        rc = results[core]["r1"]
        for t, ot in enumerate(ots1):
            full1[:, ot // 4, q1 * L1P:(q1 + 1) * L1P] += \
                rc[:, t * L1P:(t + 1) * L1P]
    o1 = full1.astype(np.float32).reshape(NT, NO, NA, BS, HW[1], HW[1])
    o1 = np.ascontiguousarray(o1.transpose(3, 0, 2, 4, 5, 1))

    full2 = np.zeros((NTJ, NA, P2F), np.float64)
    for core in range(N_CORES):
        h2, ots2 = _l2_tasks(core)
        rc = results[core]["r2"]
        for t, ot in enumerate(ots2):
            full2[:, ot // 4, h2 * L2P:(h2 + 1) * L2P] += \
                rc[:, t * L2P:(t + 1) * L2P]
    o2 = full2.astype(np.float32).reshape(NT, NO, NA, BS, HW[2], HW[2])
    o2 = np.ascontiguousarray(o2.transpose(3, 0, 2, 4, 5, 1))
    return (o0, o1, o2)


def _get_nc():
    if "nc" not in _STATE:
        _STATE["nc"] = _build()
    return _STATE["nc"]


def _run(inputs, **kw):
    nc = _get_nc()
    in_maps = _prep(inputs)
    res = run_bass_kernel_spmd(nc, in_maps, list(range(N_CORES)), **kw)
    return res


def kernel(**inputs):
    res = _run(inputs)
    return _assemble(res.results)


# revision 16
# speedup vs baseline: 1.0255x; 1.0213x over previous
"""Trainium2 Bass kernel for nn_Detect: 3-level detection head.

Math (per level, reference):
    k = conv1x1(x, k_w) + k_b          # [b, 3*512, h, w]
    v = conv1x1(x, v_w) + v_b
    kv = k * v  (flattened to [n, 512] per anchor)
    r[n, t, o] = sum_d kv[n,d] * q[t,d] * out_w[o,d] + out_b[o]
with q = target @ q_w.T + q_b.

Device strategy (8 cores, SPMD), all matmuls fp32r (fast fp32 mode):
  - L0 (x-heavy): shard pixels (h axis) 8 ways; weights replicated;
    full r on device.
  - L1: 2D shard: core = (pixel quarter, otile half) -> 6 otile tasks of
    512 pixels each; weights sharded; r partials summed on host.
  - L2 (weight-heavy): core = (pixel half, otile third) -> 3 otile tasks
    of 256 pixels; weights sharded; r partials summed on host.
  - Partial r bias: out_b/4 per otile partial (4 dchunks -> exact sum).
  - W2T[d, t*5+j] = q[t,d]*out_w[d?,j] computed on device (q GEMM on PE,
    outer products + per-task dchunk one-hot selection on GpSimd, which
    is otherwise idle -- keeps DVE free for PSUM kv evacuation).
  - PE warm-up dummies on x0 bridge the initial weight-DMA wait (HAM).
  - host does layout only: transpose/reshape/shard/unshard/partial-sum.
"""
import sys

import numpy as np

try:
    import concourse.bacc as bacc  # noqa: F401
except Exception:  # pragma: no cover
    sys.path.insert(0, "/opt/trn_rl_repo")

import concourse.bacc as bacc
import concourse.bass as bass  # noqa: F401
import concourse.tile as tile
from concourse import mybir
from concourse.bass_utils import run_bass_kernel_spmd

N_CORES = 8
NA, HD, NO, NT, TD = 3, 512, 5, 16, 512
CH = [256, 512, 1024]
HW = [64, 32, 16]
BS = 2
NTJ = NT * NO                               # 80
NB = 512                                    # matmul free-dim block

# L0: pixel shard
HS0 = HW[0] // N_CORES                      # 8 h rows per core
P0 = BS * HS0 * HW[0]                       # 1024 pixels per core

# L1: (quarter, otile-half): 6 tasks of 512 pixels
P1F = BS * HW[1] * HW[1]                    # 2048 full pixels
L1P = 512
L1T = 6

# L2: (half, otile-third): 3 tasks of 256 pixels
P2F = BS * HW[2] * HW[2]                    # 512 full pixels
L2P = 256
L2T = 3

NSLOT = L1T + L2T                           # 9 W2T task slots

# par column layout (per-core)
PC_KB0, PC_VB0 = 0, 12     # L0 biases by otile (12 each)
PC_KB1, PC_VB1 = 24, 30    # L1 per-task biases (6 each)
PC_KB2, PC_VB2 = 36, 39    # L2 per-task biases (3 each)
PC_QB = 42                 # q_b chunks (4)
PC_OW = 46                 # out_w chunks, m-major (20)
PC_OB = 66                 # out_b tiled [80]
PC_OB4 = 67                # out_b/4 tiled (partials)
PC_SEL = 68                # dchunk-selection one-hots (9 slots x 4)
PC_N = 104

MM_DT = mybir.dt.float32r
WARM_MMS = 4              # PE warm-up dummies during initial DMA wait

_STATE = {}


def _l1_tasks(core):
    q, oh = divmod(core, 2)
    return q, [oh * 6 + i for i in range(6)]


def _l2_tasks(core):
    half, og = divmod(core, 4)
    return half, [og * 3 + i for i in range(3)]


def _build():
    f32 = mybir.dt.float32
    mmd = MM_DT
    act_id = mybir.ActivationFunctionType.Identity
    nc = bacc.Bacc("TRN2", target_bir_lowering=False, debug=False,
                   num_devices=N_CORES)

    x0 = nc.dram_tensor("x0", [128, 2 * P0], mmd, kind="ExternalInput")
    w0 = nc.dram_tensor("w0", [128, 6144], mmd, kind="ExternalInput")
    x1 = nc.dram_tensor("x1", [128, 4 * L1P], mmd, kind="ExternalInput")
    w1 = nc.dram_tensor("w1", [128, 6144], mmd, kind="ExternalInput")
    x2 = nc.dram_tensor("x2", [128, 8 * L2P], mmd, kind="ExternalInput")
    w2 = nc.dram_tensor("w2", [128, 6144], mmd, kind="ExternalInput")
    qw = nc.dram_tensor("qw", [128, 4 * TD], mmd, kind="ExternalInput")
    tgt = nc.dram_tensor("tgt", [128, 4 * NT], mmd, kind="ExternalInput")
    par = nc.dram_tensor("par", [128, PC_N], f32, kind="ExternalInput")
    r0 = nc.dram_tensor("r0", [NTJ, NA * P0], f32, kind="ExternalOutput")
    r1 = nc.dram_tensor("r1", [NTJ, L1T * L1P], f32, kind="ExternalOutput")
    r2 = nc.dram_tensor("r2", [NTJ, L2T * L2P], f32, kind="ExternalOutput")

    with tile.TileContext(nc) as tc:
        with (
            tc.tile_pool(name="const", bufs=1) as cpool,
            tc.tile_pool(name="xp", bufs=3) as xpool,
            tc.tile_pool(name="w0p", bufs=3) as w0pool,
            tc.tile_pool(name="wp", bufs=2) as wpool,
            tc.tile_pool(name="vev", bufs=4) as vpool,
            tc.tile_pool(name="kv", bufs=4) as kvpool,
            tc.tile_pool(name="kvt", bufs=3) as kvtpool,
            tc.tile_pool(name="rout", bufs=4) as rpool,
            tc.tile_pool(name="ps", bufs=6, space="PSUM") as pspool,
            tc.tile_pool(name="psr", bufs=2, space="PSUM") as psrpool,
        ):
            # ---- input DMAs: ONE ring, strict consumption order ----
            x0_sb = xpool.tile([128, 2 * P0], mmd, tag="x", name="x0_sb")
            w0_sb = [w0pool.tile([128, 2048], mmd, tag="w0", name=f"w0_{g}")
                     for g in range(3)]
            par_sb = cpool.tile([128, PC_N], f32, tag="par")
            tgt_sb = cpool.tile([128, 4 * NT], mmd, tag="tgt")
            qw_sb = cpool.tile([128, 4 * TD], mmd, tag="qw")
            x1_sb = xpool.tile([128, 4 * L1P], mmd, tag="x", name="x1_sb")
            w1_sb = wpool.tile([128, 6144], mmd, tag="w", name="w1_sb")
            x2_sb = xpool.tile([128, 8 * L2P], mmd, tag="x", name="x2_sb")
            w2_sb = wpool.tile([128, 6144], mmd, tag="w", name="w2_sb")
            nc.sync.dma_start(x0_sb[:], x0.ap())
            nc.sync.dma_start(w0_sb[0][:], w0.ap()[:, 0:2048])
            nc.sync.dma_start(par_sb[:], par.ap())
            nc.sync.dma_start(tgt_sb[:], tgt.ap())
            nc.sync.dma_start(w0_sb[1][:], w0.ap()[:, 2048:4096])
            nc.sync.dma_start(w0_sb[2][:], w0.ap()[:, 4096:6144])
            nc.sync.dma_start(qw_sb[:], qw.ap())
            nc.sync.dma_start(x1_sb[:], x1.ap())
            nc.sync.dma_start(w1_sb[:], w1.ap())
            nc.sync.dma_start(x2_sb[:], x2.ap())
            nc.sync.dma_start(w2_sb[:], w2.ap())

            qT_sb = cpool.tile([128, 4 * NT], f32, tag="qT")

            def emit_q_gemm():
                # q = target @ q_w.T + q_b, computed as qT [512(hd), 16]
                for m in range(4):
                    psq = psrpool.tile([128, NB], f32, tag="psr")
                    for cc in range(4):
                        nc.tensor.matmul(
                            psq[:, :NT],
                            qw_sb[:, cc * TD + m * 128:
                                  cc * TD + (m + 1) * 128],
                            tgt_sb[:, cc * NT:(cc + 1) * NT],
                            start=(cc == 0), stop=(cc == 3),
                        )
                    nc.scalar.activation(
                        qT_sb[:, m * NT:(m + 1) * NT], psq[:, :NT], act_id,
                        bias=par_sb[:, PC_QB + m:PC_QB + m + 1],
                    )

            def emit_warmups():
                # PE warm-up dummies on x0 (bridge the w0 DMA wait)
                for i in range(WARM_MMS):
                    wps = psrpool.tile([128, NB], f32, tag="psr")
                    nc.tensor.matmul(
                        wps[:], x0_sb[:, (i % 8) * 128:(i % 8) * 128 + 128],
                        x0_sb[:, 0:NB], start=True, stop=True)
                    if i % 4 == 3:
                        wsc = vpool.tile([128, NB], f32, tag="vev")
                        nc.scalar.activation(wsc[:, 0:1], wps[:, 0:1],
                                             act_id, bias=0.0)

            # ---- W2T formation (emitted into DVE idle slots mid-conv) ----
            w2f_sb = cpool.tile([128, 4 * NTJ], mmd, tag="w2f")
            w2f_4d = w2f_sb[:].rearrange("p (m t j) -> p m t j",
                                         m=4, t=NT, j=NO)

            def emit_w2f():
                for m in range(4):
                    for j in range(NO):
                        nc.vector.tensor_scalar_mul(
                            w2f_4d[:, m, :, j],
                            qT_sb[:, m * NT:(m + 1) * NT],
                            par_sb[:, PC_OW + m * NO + j:
                                   PC_OW + m * NO + j + 1],
                        )
            # per-slot dchunk selection (w2task[s] = sum_m w2f[m]*sel[s,m])
            # is emitted inside the task loops to sit in DVE idle slots
            w2task_sb = cpool.tile([128, NSLOT * NTJ], mmd, tag="w2task")

            def emit_w2task_select(s):
                o = s * NTJ
                nc.vector.tensor_scalar_mul(
                    w2task_sb[:, o:o + NTJ], w2f_sb[:, 0:NTJ],
                    par_sb[:, PC_SEL + s * 4:PC_SEL + s * 4 + 1])
                for m in range(1, 4):
                    nc.vector.scalar_tensor_tensor(
                        w2task_sb[:, o:o + NTJ],
                        w2f_sb[:, m * NTJ:(m + 1) * NTJ],
                        par_sb[:, PC_SEL + s * 4 + m:PC_SEL + s * 4 + m + 1],
                        w2task_sb[:, o:o + NTJ],
                        op0=mybir.AluOpType.mult,
                        op1=mybir.AluOpType.add,
                    )

            # ---- L0: pixel-sharded, full r on device ----
            emit_warmups()
            kv_sb = [kvpool.tile([128, NA * P0], mmd, tag="kv",
                                 name=f"kv_d{d}") for d in range(4)]
            for ot in range(12):            # otile = a*4 + dchunk
                if ot == 4:
                    emit_q_gemm()
                a, dchunk = divmod(ot, 4)
                g, lot = divmod(ot, 4)      # w0 third g, local otile
                for pb in range(2):
                    psk = pspool.tile([128, NB], f32, tag="psc")
                    psv = pspool.tile([128, NB], f32, tag="psc")
                    for cc in range(2):
                        nc.tensor.matmul(
                            psk[:],
                            w0_sb[g][:, lot * 512 + cc * 128:
                                   lot * 512 + (cc + 1) * 128],
                            x0_sb[:, cc * P0 + pb * NB:cc * P0 + pb * NB + NB],
                            start=(cc == 0), stop=(cc == 1),
                        )
                    for cc in range(2):
                        nc.tensor.matmul(
                            psv[:],
                            w0_sb[g][:, lot * 512 + 256 + cc * 128:
                                   lot * 512 + 256 + (cc + 1) * 128],
                            x0_sb[:, cc * P0 + pb * NB:cc * P0 + pb * NB + NB],
                            start=(cc == 0), stop=(cc == 1),
                        )
                    v_sb = vpool.tile([128, NB], f32, tag="vev")
                    nc.scalar.activation(
                        v_sb[:], psv[:], act_id,
                        bias=par_sb[:, PC_VB0 + ot:PC_VB0 + ot + 1],
                    )
                    nc.vector.scalar_tensor_tensor(
                        kv_sb[dchunk][:, a * P0 + pb * NB:
                                      a * P0 + pb * NB + NB],
                        psk[:],
                        par_sb[:, PC_KB0 + ot:PC_KB0 + ot + 1],
                        v_sb[:],
                        op0=mybir.AluOpType.add,
                        op1=mybir.AluOpType.mult,
                    )
                if ot == 5:
                    emit_w2f()
                if ot == 8:
                    for _s in range(NSLOT):
                        emit_w2task_select(_s)
            for nb_i in range(NA * P0 // NB):       # 6 r blocks
                psr = psrpool.tile([NTJ, NB], f32, tag="psr")
                for dchunk in range(4):
                    nc.tensor.matmul(
                        psr[:],
                        w2f_sb[:, dchunk * NTJ:(dchunk + 1) * NTJ],
                        kv_sb[dchunk][:, nb_i * NB:nb_i * NB + NB],
                        start=(dchunk == 0), stop=(dchunk == 3),
                    )
                r_sb = rpool.tile([NTJ, NB], f32, tag="rout")
                nc.scalar.activation(
                    r_sb[:], psr[:], act_id,
                    bias=par_sb[:NTJ, PC_OB:PC_OB + 1],
                )
                nc.gpsimd.dma_start(
                    r0.ap()[:, nb_i * NB:nb_i * NB + NB], r_sb[:])

            # ---- L1 (6 tasks x 512 px, 4 cc), L2 (3 tasks x 256 px, 8 cc):
            #      otile-task partials ----
            for lvl in (1, 2):
                ntask = L1T if lvl == 1 else L2T
                npx = L1P if lvl == 1 else L2P
                ncc = 4 if lvl == 1 else 8
                xsb = x1_sb if lvl == 1 else x2_sb
                wsb = w1_sb if lvl == 1 else w2_sb
                rdram = r1 if lvl == 1 else r2
                pckb = PC_KB1 if lvl == 1 else PC_KB2
                pcvb = PC_VB1 if lvl == 1 else PC_VB2
                slot0 = 0 if lvl == 1 else L1T
                for t in range(ntask):
                    psk = pspool.tile([128, NB], f32, tag="psc")
                    psv = pspool.tile([128, NB], f32, tag="psc")
                    kb = (t * 2 + 0) * ncc * 128
                    vb = (t * 2 + 1) * ncc * 128
                    for cc in range(ncc):
                        nc.tensor.matmul(
                            psk[:, :npx],
                            wsb[:, kb + cc * 128:kb + (cc + 1) * 128],
                            xsb[:, cc * npx:(cc + 1) * npx],
                            start=(cc == 0), stop=(cc == ncc - 1),
                        )
                    for cc in range(ncc):
                        nc.tensor.matmul(
                            psv[:, :npx],
                            wsb[:, vb + cc * 128:vb + (cc + 1) * 128],
                            xsb[:, cc * npx:(cc + 1) * npx],
                            start=(cc == 0), stop=(cc == ncc - 1),
                        )
                    v_sb = vpool.tile([128, NB], f32, tag="vev")
                    nc.scalar.activation(
                        v_sb[:, :npx], psv[:, :npx], act_id,
                        bias=par_sb[:, pcvb + t:pcvb + t + 1],
                    )
                    kv_t = kvtpool.tile([128, NB], mmd, tag="kvt")
                    nc.vector.scalar_tensor_tensor(
                        kv_t[:, :npx], psk[:, :npx],
                        par_sb[:, pckb + t:pckb + t + 1],
                        v_sb[:, :npx],
                        op0=mybir.AluOpType.add,
                        op1=mybir.AluOpType.mult,
                    )
                    s = slot0 + t
                    psr = psrpool.tile([NTJ, NB], f32, tag="psr")
                    nc.tensor.matmul(
                        psr[:, :npx],
                        w2task_sb[:, s * NTJ:(s + 1) * NTJ],
                        kv_t[:, :npx],
                        start=True, stop=True,
                    )
                    r_sb = rpool.tile([NTJ, NB], f32, tag="rout")
                    nc.scalar.activation(
                        r_sb[:, :npx], psr[:, :npx], act_id,
                        bias=par_sb[:NTJ, PC_OB4:PC_OB4 + 1],
                    )
                    nc.gpsimd.dma_start(
                        rdram.ap()[:, t * npx:(t + 1) * npx], r_sb[:, :npx])

    nc.compile()
    return nc


def _chunk128(arr):
    """[C, F] -> [128, (C//128)*F] with chunk-major columns."""
    c, f = arr.shape
    return np.ascontiguousarray(
        arr.reshape(c // 128, 128, f).transpose(1, 0, 2).reshape(128, -1))


def _prep(inputs):
    """Host-side layout prep. Returns per-core input maps."""
    mm_np = mybir.dt.np(MM_DT)
    f = lambda k: np.asarray(inputs[k], dtype=np.float32)

    shared = {}
    # L0 weights, replicated: col = g*2048 + lot*512 + kv*256 + cc*128
    kwT0, vwT0 = f("k_w0").T, f("v_w0").T     # [256, 1536]
    w0cols = np.empty((128, 6144), np.float32)
    for ot in range(12):
        g, lot = divmod(ot, 4)
        base = g * 2048 + lot * 512
        w0cols[:, base:base + 256] = _chunk128(kwT0[:, ot * 128:(ot + 1) * 128])
        w0cols[:, base + 256:base + 512] = \
            _chunk128(vwT0[:, ot * 128:(ot + 1) * 128])
    shared["w0"] = w0cols.astype(mm_np)
    shared["qw"] = _chunk128(f("q_w").T).astype(mm_np)
    shared["tgt"] = _chunk128(f("target").T).astype(mm_np)

    kwT1, vwT1 = f("k_w1").T, f("v_w1").T     # [512, 1536]
    kwT2, vwT2 = f("k_w2").T, f("v_w2").T     # [1024, 1536]
    kb1 = f("k_b1").reshape(12, 128)
    vb1 = f("v_b1").reshape(12, 128)
    kb2 = f("k_b2").reshape(12, 128)
    vb2 = f("v_b2").reshape(12, 128)
    ow128 = _chunk128(f("out_w").T)           # [128, 20] m-major

    par_base = np.zeros((128, PC_N), np.float32)
    par_base[:, PC_KB0:PC_KB0 + 12] = f("k_b0").reshape(12, 128).T
    par_base[:, PC_VB0:PC_VB0 + 12] = f("v_b0").reshape(12, 128).T
    par_base[:, PC_QB:PC_QB + 4] = f("q_b").reshape(4, 128).T
    par_base[:, PC_OW:PC_OW + 20] = ow128
    par_base[:NTJ, PC_OB] = np.tile(f("out_b"), NT)
    par_base[:NTJ, PC_OB4] = np.tile(f("out_b"), NT) / 4.0

    xt1 = np.ascontiguousarray(
        f("x1").transpose(1, 0, 2, 3)).reshape(CH[1], -1)   # [512, 2048]
    xt2 = np.ascontiguousarray(
        f("x2").transpose(1, 0, 2, 3)).reshape(CH[2], -1)   # [1024, 512]

    in_maps = []
    for core in range(N_CORES):
        m = dict(shared)
        x = f("x0")[:, :, core * HS0:(core + 1) * HS0, :]
        xt = np.ascontiguousarray(x.transpose(1, 0, 2, 3)).reshape(CH[0], -1)
        m["x0"] = _chunk128(xt).astype(mm_np)

        par = par_base.copy()

        q1, ots1 = _l1_tasks(core)
        m["x1"] = _chunk128(xt1[:, q1 * L1P:(q1 + 1) * L1P]).astype(mm_np)
        w1cols = np.empty((128, 6144), np.float32)
        for t, ot in enumerate(ots1):
            dchunk = ot % 4
            par[:, PC_KB1 + t] = kb1[ot]
            par[:, PC_VB1 + t] = vb1[ot]
            par[:, PC_SEL + t * 4 + dchunk] = 1.0
            w1cols[:, (t * 2) * 512:(t * 2 + 1) * 512] = \
                _chunk128(kwT1[:, ot * 128:(ot + 1) * 128])
            w1cols[:, (t * 2 + 1) * 512:(t * 2 + 2) * 512] = \
                _chunk128(vwT1[:, ot * 128:(ot + 1) * 128])
        m["w1"] = w1cols.astype(mm_np)

        h2, ots2 = _l2_tasks(core)
        m["x2"] = _chunk128(xt2[:, h2 * L2P:(h2 + 1) * L2P]).astype(mm_np)
        w2cols = np.empty((128, 6144), np.float32)
        for t, ot in enumerate(ots2):
            s = L1T + t
            dchunk = ot % 4
            par[:, PC_KB2 + t] = kb2[ot]
            par[:, PC_VB2 + t] = vb2[ot]
            par[:, PC_SEL + s * 4 + dchunk] = 1.0
            w2cols[:, (t * 2) * 1024:(t * 2 + 1) * 1024] = \
                _chunk128(kwT2[:, ot * 128:(ot + 1) * 128])
            w2cols[:, (t * 2 + 1) * 1024:(t * 2 + 2) * 1024] = \
                _chunk128(vwT2[:, ot * 128:(ot + 1) * 128])
        m["w2"] = w2cols.astype(mm_np)

        m["par"] = par
        in_maps.append(m)
    return in_maps


def _assemble(results):
    """Per-core outputs -> tuple of 3 full [2,16,3,h,w,5] arrays."""
    parts = []
    for core in range(N_CORES):
        r = results[core]["r0"].reshape(NT, NO, NA, BS, HS0, HW[0])
        parts.append(r.transpose(3, 0, 2, 4, 5, 1))
    o0 = np.ascontiguousarray(np.concatenate(parts, axis=3))

    full1 = np.zeros((NTJ, NA, P1F), np.float64)
    for core in range(N_CORES):
        q1, ots1 = _l1_tasks(core)
        rc = results[core]["r1"]
        for t, ot in enumerate(ots1):
            full1[:, ot // 4, q1 * L1P:(q1 + 1) * L1P] += \
                rc[:, t * L1P:(t + 1) * L1P]
    o1 = full1.astype(np.float32).reshape(NT, NO, NA, BS, HW[1], HW[1])
    o1 = np.ascontiguousarray(o1.transpose(3, 0, 2, 4, 5, 1))

    full2 = np.zeros((NTJ, NA, P2F), np.float64)
    for core in range(N_CORES):
        h2, ots2 = _l2_tasks(core)
        rc = results[core]["r2"]
        for t, ot in enumerate(ots2):
            full2[:, ot // 4, h2 * L2P:(h2 + 1) * L2P] += \
                rc[:, t * L2P:(t + 1) * L2P]
    o2 = full2.astype(np.float32).reshape(NT, NO, NA, BS, HW[2], HW[2])
    o2 = np.ascontiguousarray(o2.transpose(3, 0, 2, 4, 5, 1))
    return (o0, o1, o2)


def _get_nc():
    if "nc" not in _STATE:
        _STATE["nc"] = _build()
    return _STATE["nc"]


def _run(inputs, **kw):
    nc = _get_nc()
    in_maps = _prep(inputs)
    res = run_bass_kernel_spmd(nc, in_maps, list(range(N_CORES)), **kw)
    return res


def kernel(**inputs):
    res = _run(inputs)
    return _assemble(res.results)


# revision 17
# speedup vs baseline: 1.1130x; 1.0853x over previous
"""Trainium2 Bass kernel for nn_Detect: 3-level detection head.

Math (per level, reference):
    k = conv1x1(x, k_w) + k_b          # [b, 3*512, h, w]
    v = conv1x1(x, v_w) + v_b
    kv = k * v  (flattened to [n, 512] per anchor)
    r[n, t, o] = sum_d kv[n,d] * q[t,d] * out_w[o,d] + out_b[o]
with q = target @ q_w.T + q_b.

Device strategy (8 cores, SPMD), all matmuls fp32r (fast fp32 mode):
  - L0 (x-heavy): shard pixels (h axis) 8 ways; weights replicated;
    full r on device.
  - L1: 2D shard: core = (pixel quarter, otile half) -> 6 otile tasks of
    512 pixels each; weights sharded; r partials summed on host.
  - L2 (weight-heavy): core = (pixel half, otile third) -> 3 otile tasks
    of 256 pixels; weights sharded; r partials summed on host.
  - Partial r bias: out_b/4 per otile partial (4 dchunks -> exact sum).
  - W2T[d, t*5+j] = q[t,d]*out_w[d?,j] computed on device (q GEMM on PE,
    outer products + per-task dchunk one-hot selection on GpSimd, which
    is otherwise idle -- keeps DVE free for PSUM kv evacuation).
  - PE warm-up dummies on x0 bridge the initial weight-DMA wait (HAM).
  - host does layout only: transpose/reshape/shard/unshard/partial-sum.
"""
import sys

import numpy as np

try:
    import concourse.bacc as bacc  # noqa: F401
except Exception:  # pragma: no cover
    sys.path.insert(0, "/opt/trn_rl_repo")

import concourse.bacc as bacc
import concourse.bass as bass  # noqa: F401
import concourse.tile as tile
from concourse import mybir
from concourse.bass_utils import run_bass_kernel_spmd

N_CORES = 8
NA, HD, NO, NT, TD = 3, 512, 5, 16, 512
CH = [256, 512, 1024]
HW = [64, 32, 16]
BS = 2
NTJ = NT * NO                               # 80
NB = 512                                    # matmul free-dim block

# L0: pixel shard
HS0 = HW[0] // N_CORES                      # 8 h rows per core
P0 = BS * HS0 * HW[0]                       # 1024 pixels per core

# L1: (quarter, otile-half): 6 tasks of 512 pixels
P1F = BS * HW[1] * HW[1]                    # 2048 full pixels
L1P = 512
L1T = 6

# L2: (half, otile-third): 3 tasks of 256 pixels
P2F = BS * HW[2] * HW[2]                    # 512 full pixels
L2P = 256
L2T = 3

NSLOT = L1T + L2T                           # 9 W2T task slots

# par column layout (per-core)
PC_KB0, PC_VB0 = 0, 12     # L0 biases by otile (12 each)
PC_KB1, PC_VB1 = 24, 30    # L1 per-task biases (6 each)
PC_KB2, PC_VB2 = 36, 39    # L2 per-task biases (3 each)
PC_QB = 42                 # q_b chunks (4)
PC_OW = 46                 # out_w chunks, m-major (20)
PC_OB = 66                 # out_b tiled [80]
PC_OB4 = 67                # out_b/4 tiled (partials)
PC_SEL = 68                # dchunk-selection one-hots (9 slots x 4)
PC_N = 104

MM_DT = mybir.dt.bfloat16
WARM_MMS = 4              # PE warm-up dummies during initial DMA wait

_STATE = {}


def _l1_tasks(core):
    q, oh = divmod(core, 2)
    return q, [oh * 6 + i for i in range(6)]


def _l2_tasks(core):
    half, og = divmod(core, 4)
    return half, [og * 3 + i for i in range(3)]


def _build():
    f32 = mybir.dt.float32
    mmd = MM_DT
    act_id = mybir.ActivationFunctionType.Identity
    nc = bacc.Bacc("TRN2", target_bir_lowering=False, debug=False,
                   num_devices=N_CORES)

    x0 = nc.dram_tensor("x0", [128, 2 * P0], mmd, kind="ExternalInput")
    w0 = nc.dram_tensor("w0", [128, 6144], mmd, kind="ExternalInput")
    x1 = nc.dram_tensor("x1", [128, 4 * L1P], mmd, kind="ExternalInput")
    w1 = nc.dram_tensor("w1", [128, 6144], mmd, kind="ExternalInput")
    x2 = nc.dram_tensor("x2", [128, 8 * L2P], mmd, kind="ExternalInput")
    w2 = nc.dram_tensor("w2", [128, 6144], mmd, kind="ExternalInput")
    qw = nc.dram_tensor("qw", [128, 4 * TD], mmd, kind="ExternalInput")
    tgt = nc.dram_tensor("tgt", [128, 4 * NT], mmd, kind="ExternalInput")
    par = nc.dram_tensor("par", [128, PC_N], f32, kind="ExternalInput")
    r0 = nc.dram_tensor("r0", [NTJ, NA * P0], f32, kind="ExternalOutput")
    r1 = nc.dram_tensor("r1", [NTJ, L1T * L1P], f32, kind="ExternalOutput")
    r2 = nc.dram_tensor("r2", [NTJ, L2T * L2P], f32, kind="ExternalOutput")

    with tile.TileContext(nc) as tc:
        with (
            tc.tile_pool(name="const", bufs=1) as cpool,
            tc.tile_pool(name="xp", bufs=3) as xpool,
            tc.tile_pool(name="w0p", bufs=3) as w0pool,
            tc.tile_pool(name="wp", bufs=2) as wpool,
            tc.tile_pool(name="vev", bufs=4) as vpool,
            tc.tile_pool(name="kv", bufs=4) as kvpool,
            tc.tile_pool(name="kvt", bufs=3) as kvtpool,
            tc.tile_pool(name="rout", bufs=4) as rpool,
            tc.tile_pool(name="ps", bufs=6, space="PSUM") as pspool,
            tc.tile_pool(name="psr", bufs=2, space="PSUM") as psrpool,
        ):
            # ---- input DMAs: ONE ring, strict consumption order ----
            x0_sb = xpool.tile([128, 2 * P0], mmd, tag="x", name="x0_sb")
            w0_sb = [w0pool.tile([128, 2048], mmd, tag="w0", name=f"w0_{g}")
                     for g in range(3)]
            par_sb = cpool.tile([128, PC_N], f32, tag="par")
            tgt_sb = cpool.tile([128, 4 * NT], mmd, tag="tgt")
            qw_sb = cpool.tile([128, 4 * TD], mmd, tag="qw")
            x1_sb = xpool.tile([128, 4 * L1P], mmd, tag="x", name="x1_sb")
            w1_sb = wpool.tile([128, 6144], mmd, tag="w", name="w1_sb")
            x2_sb = xpool.tile([128, 8 * L2P], mmd, tag="x", name="x2_sb")
            w2_sb = wpool.tile([128, 6144], mmd, tag="w", name="w2_sb")
            nc.sync.dma_start(x0_sb[:], x0.ap())
            nc.sync.dma_start(w0_sb[0][:], w0.ap()[:, 0:2048])
            nc.sync.dma_start(par_sb[:], par.ap())
            nc.sync.dma_start(tgt_sb[:], tgt.ap())
            nc.sync.dma_start(w0_sb[1][:], w0.ap()[:, 2048:4096])
            nc.sync.dma_start(w0_sb[2][:], w0.ap()[:, 4096:6144])
            nc.sync.dma_start(qw_sb[:], qw.ap())
            nc.sync.dma_start(x1_sb[:], x1.ap())
            nc.sync.dma_start(w1_sb[:], w1.ap())
            nc.sync.dma_start(x2_sb[:], x2.ap())
            nc.sync.dma_start(w2_sb[:], w2.ap())

            qT_sb = cpool.tile([128, 4 * NT], f32, tag="qT")

            def emit_q_gemm():
                # q = target @ q_w.T + q_b, computed as qT [512(hd), 16]
                for m in range(4):
                    psq = psrpool.tile([128, NB], f32, tag="psr")
                    for cc in range(4):
                        nc.tensor.matmul(
                            psq[:, :NT],
                            qw_sb[:, cc * TD + m * 128:
                                  cc * TD + (m + 1) * 128],
                            tgt_sb[:, cc * NT:(cc + 1) * NT],
                            start=(cc == 0), stop=(cc == 3),
                        )
                    nc.scalar.activation(
                        qT_sb[:, m * NT:(m + 1) * NT], psq[:, :NT], act_id,
                        bias=par_sb[:, PC_QB + m:PC_QB + m + 1],
                    )

            def emit_warmups():
                # PE warm-up dummies on x0 (bridge the w0 DMA wait)
                for i in range(WARM_MMS):
                    wps = psrpool.tile([128, NB], f32, tag="psr")
                    nc.tensor.matmul(
                        wps[:], x0_sb[:, (i % 8) * 128:(i % 8) * 128 + 128],
                        x0_sb[:, 0:NB], start=True, stop=True)
                    if i % 4 == 3:
                        wsc = vpool.tile([128, NB], f32, tag="vev")
                        nc.scalar.activation(wsc[:, 0:1], wps[:, 0:1],
                                             act_id, bias=0.0)

            # ---- W2T formation (emitted into DVE idle slots mid-conv) ----
            w2f_sb = cpool.tile([128, 4 * NTJ], mmd, tag="w2f")
            w2f_4d = w2f_sb[:].rearrange("p (m t j) -> p m t j",
                                         m=4, t=NT, j=NO)

            def emit_w2f():
                for m in range(4):
                    for j in range(NO):
                        nc.vector.tensor_scalar_mul(
                            w2f_4d[:, m, :, j],
                            qT_sb[:, m * NT:(m + 1) * NT],
                            par_sb[:, PC_OW + m * NO + j:
                                   PC_OW + m * NO + j + 1],
                        )
            # per-slot dchunk selection (w2task[s] = sum_m w2f[m]*sel[s,m])
            # is emitted inside the task loops to sit in DVE idle slots
            w2task_sb = cpool.tile([128, NSLOT * NTJ], mmd, tag="w2task")

            def emit_w2task_select(s):
                o = s * NTJ
                nc.vector.tensor_scalar_mul(
                    w2task_sb[:, o:o + NTJ], w2f_sb[:, 0:NTJ],
                    par_sb[:, PC_SEL + s * 4:PC_SEL + s * 4 + 1])
                for m in range(1, 4):
                    nc.vector.scalar_tensor_tensor(
                        w2task_sb[:, o:o + NTJ],
                        w2f_sb[:, m * NTJ:(m + 1) * NTJ],
                        par_sb[:, PC_SEL + s * 4 + m:PC_SEL + s * 4 + m + 1],
                        w2task_sb[:, o:o + NTJ],
                        op0=mybir.AluOpType.mult,
                        op1=mybir.AluOpType.add,
                    )

            # ---- L0: pixel-sharded, full r on device ----
            emit_warmups()
            kv_sb = [kvpool.tile([128, NA * P0], mmd, tag="kv",
                                 name=f"kv_d{d}") for d in range(4)]
            for ot in range(12):            # otile = a*4 + dchunk
                if ot == 4:
                    emit_q_gemm()
                a, dchunk = divmod(ot, 4)
                g, lot = divmod(ot, 4)      # w0 third g, local otile
                for pb in range(2):
                    psk = pspool.tile([128, NB], f32, tag="psc")
                    psv = pspool.tile([128, NB], f32, tag="psc")
                    for cc in range(2):
                        nc.tensor.matmul(
                            psk[:],
                            w0_sb[g][:, lot * 512 + cc * 128:
                                   lot * 512 + (cc + 1) * 128],
                            x0_sb[:, cc * P0 + pb * NB:cc * P0 + pb * NB + NB],
                            start=(cc == 0), stop=(cc == 1),
                        )
                    for cc in range(2):
                        nc.tensor.matmul(
                            psv[:],
                            w0_sb[g][:, lot * 512 + 256 + cc * 128:
                                   lot * 512 + 256 + (cc + 1) * 128],
                            x0_sb[:, cc * P0 + pb * NB:cc * P0 + pb * NB + NB],
                            start=(cc == 0), stop=(cc == 1),
                        )
                    v_sb = vpool.tile([128, NB], f32, tag="vev")
                    nc.scalar.activation(
                        v_sb[:], psv[:], act_id,
                        bias=par_sb[:, PC_VB0 + ot:PC_VB0 + ot + 1],
                    )
                    nc.vector.scalar_tensor_tensor(
                        kv_sb[dchunk][:, a * P0 + pb * NB:
                                      a * P0 + pb * NB + NB],
                        psk[:],
                        par_sb[:, PC_KB0 + ot:PC_KB0 + ot + 1],
                        v_sb[:],
                        op0=mybir.AluOpType.add,
                        op1=mybir.AluOpType.mult,
                    )
                if ot == 5:
                    emit_w2f()
                if ot == 8:
                    for _s in range(NSLOT):
                        emit_w2task_select(_s)
            for nb_i in range(NA * P0 // NB):       # 6 r blocks
                psr = psrpool.tile([NTJ, NB], f32, tag="psr")
                for dchunk in range(4):
                    nc.tensor.matmul(
                        psr[:],
                        w2f_sb[:, dchunk * NTJ:(dchunk + 1) * NTJ],
                        kv_sb[dchunk][:, nb_i * NB:nb_i * NB + NB],
                        start=(dchunk == 0), stop=(dchunk == 3),
                    )
                r_sb = rpool.tile([NTJ, NB], f32, tag="rout")
                nc.scalar.activation(
                    r_sb[:], psr[:], act_id,
                    bias=par_sb[:NTJ, PC_OB:PC_OB + 1],
                )
                nc.gpsimd.dma_start(
                    r0.ap()[:, nb_i * NB:nb_i * NB + NB], r_sb[:])

            # ---- L1 (6 tasks x 512 px, 4 cc), L2 (3 tasks x 256 px, 8 cc):
            #      otile-task partials ----
            for lvl in (1, 2):
                ntask = L1T if lvl == 1 else L2T
                npx = L1P if lvl == 1 else L2P
                ncc = 4 if lvl == 1 else 8
                xsb = x1_sb if lvl == 1 else x2_sb
                wsb = w1_sb if lvl == 1 else w2_sb
                rdram = r1 if lvl == 1 else r2
                pckb = PC_KB1 if lvl == 1 else PC_KB2
                pcvb = PC_VB1 if lvl == 1 else PC_VB2
                slot0 = 0 if lvl == 1 else L1T
                for t in range(ntask):
                    psk = pspool.tile([128, NB], f32, tag="psc")
                    psv = pspool.tile([128, NB], f32, tag="psc")
                    kb = (t * 2 + 0) * ncc * 128
                    vb = (t * 2 + 1) * ncc * 128
                    for cc in range(ncc):
                        nc.tensor.matmul(
                            psk[:, :npx],
                            wsb[:, kb + cc * 128:kb + (cc + 1) * 128],
                            xsb[:, cc * npx:(cc + 1) * npx],
                            start=(cc == 0), stop=(cc == ncc - 1),
                        )
                    for cc in range(ncc):
                        nc.tensor.matmul(
                            psv[:, :npx],
                            wsb[:, vb + cc * 128:vb + (cc + 1) * 128],
                            xsb[:, cc * npx:(cc + 1) * npx],
                            start=(cc == 0), stop=(cc == ncc - 1),
                        )
                    v_sb = vpool.tile([128, NB], f32, tag="vev")
                    nc.scalar.activation(
                        v_sb[:, :npx], psv[:, :npx], act_id,
                        bias=par_sb[:, pcvb + t:pcvb + t + 1],
                    )
                    kv_t = kvtpool.tile([128, NB], mmd, tag="kvt")
                    nc.vector.scalar_tensor_tensor(
                        kv_t[:, :npx], psk[:, :npx],
                        par_sb[:, pckb + t:pckb + t + 1],
                        v_sb[:, :npx],
                        op0=mybir.AluOpType.add,
                        op1=mybir.AluOpType.mult,
                    )
                    s = slot0 + t
                    psr = psrpool.tile([NTJ, NB], f32, tag="psr")
                    nc.tensor.matmul(
                        psr[:, :npx],
                        w2task_sb[:, s * NTJ:(s + 1) * NTJ],
                        kv_t[:, :npx],
                        start=True, stop=True,
                    )
                    r_sb = rpool.tile([NTJ, NB], f32, tag="rout")
                    nc.scalar.activation(
                        r_sb[:, :npx], psr[:, :npx], act_id,
                        bias=par_sb[:NTJ, PC_OB4:PC_OB4 + 1],
                    )
                    nc.gpsimd.dma_start(
                        rdram.ap()[:, t * npx:(t + 1) * npx], r_sb[:, :npx])

    nc.compile()
    return nc


def _chunk128(arr):
    """[C, F] -> [128, (C//128)*F] with chunk-major columns."""
    c, f = arr.shape
    return np.ascontiguousarray(
        arr.reshape(c // 128, 128, f).transpose(1, 0, 2).reshape(128, -1))


def _prep(inputs):
    """Host-side layout prep. Returns per-core input maps."""
    mm_np = mybir.dt.np(MM_DT)
    f = lambda k: np.asarray(inputs[k], dtype=np.float32)

    shared = {}
    # L0 weights, replicated: col = g*2048 + lot*512 + kv*256 + cc*128
    kwT0, vwT0 = f("k_w0").T, f("v_w0").T     # [256, 1536]
    w0cols = np.empty((128, 6144), np.float32)
    for ot in range(12):
        g, lot = divmod(ot, 4)
        base = g * 2048 + lot * 512
        w0cols[:, base:base + 256] = _chunk128(kwT0[:, ot * 128:(ot + 1) * 128])
        w0cols[:, base + 256:base + 512] = \
            _chunk128(vwT0[:, ot * 128:(ot + 1) * 128])
    shared["w0"] = w0cols.astype(mm_np)
    shared["qw"] = _chunk128(f("q_w").T).astype(mm_np)
    shared["tgt"] = _chunk128(f("target").T).astype(mm_np)

    kwT1, vwT1 = f("k_w1").T, f("v_w1").T     # [512, 1536]
    kwT2, vwT2 = f("k_w2").T, f("v_w2").T     # [1024, 1536]
    kb1 = f("k_b1").reshape(12, 128)
    vb1 = f("v_b1").reshape(12, 128)
    kb2 = f("k_b2").reshape(12, 128)
    vb2 = f("v_b2").reshape(12, 128)
    ow128 = _chunk128(f("out_w").T)           # [128, 20] m-major

    par_base = np.zeros((128, PC_N), np.float32)
    par_base[:, PC_KB0:PC_KB0 + 12] = f("k_b0").reshape(12, 128).T
    par_base[:, PC_VB0:PC_VB0 + 12] = f("v_b0").reshape(12, 128).T
    par_base[:, PC_QB:PC_QB + 4] = f("q_b").reshape(4, 128).T
    par_base[:, PC_OW:PC_OW + 20] = ow128
    par_base[:NTJ, PC_OB] = np.tile(f("out_b"), NT)
    par_base[:NTJ, PC_OB4] = np.tile(f("out_b"), NT) / 4.0

    xt1 = np.ascontiguousarray(
        f("x1").transpose(1, 0, 2, 3)).reshape(CH[1], -1)   # [512, 2048]
    xt2 = np.ascontiguousarray(
        f("x2").transpose(1, 0, 2, 3)).reshape(CH[2], -1)   # [1024, 512]

    in_maps = []
    for core in range(N_CORES):
        m = dict(shared)
        x = f("x0")[:, :, core * HS0:(core + 1) * HS0, :]
        xt = np.ascontiguousarray(x.transpose(1, 0, 2, 3)).reshape(CH[0], -1)
        m["x0"] = _chunk128(xt).astype(mm_np)

        par = par_base.copy()

        q1, ots1 = _l1_tasks(core)
        m["x1"] = _chunk128(xt1[:, q1 * L1P:(q1 + 1) * L1P]).astype(mm_np)
        w1cols = np.empty((128, 6144), np.float32)
        for t, ot in enumerate(ots1):
            dchunk = ot % 4
            par[:, PC_KB1 + t] = kb1[ot]
            par[:, PC_VB1 + t] = vb1[ot]
            par[:, PC_SEL + t * 4 + dchunk] = 1.0
            w1cols[:, (t * 2) * 512:(t * 2 + 1) * 512] = \
                _chunk128(kwT1[:, ot * 128:(ot + 1) * 128])
            w1cols[:, (t * 2 + 1) * 512:(t * 2 + 2) * 512] = \
                _chunk128(vwT1[:, ot * 128:(ot + 1) * 128])
        m["w1"] = w1cols.astype(mm_np)

        h2, ots2 = _l2_tasks(core)
        m["x2"] = _chunk128(xt2[:, h2 * L2P:(h2 + 1) * L2P]).astype(mm_np)
        w2cols = np.empty((128, 6144), np.float32)
        for t, ot in enumerate(ots2):
            s = L1T + t
            dchunk = ot % 4
            par[:, PC_KB2 + t] = kb2[ot]
            par[:, PC_VB2 + t] = vb2[ot]
            par[:, PC_SEL + s * 4 + dchunk] = 1.0
            w2cols[:, (t * 2) * 1024:(t * 2 + 1) * 1024] = \
                _chunk128(kwT2[:, ot * 128:(ot + 1) * 128])
            w2cols[:, (t * 2 + 1) * 1024:(t * 2 + 2) * 1024] = \
                _chunk128(vwT2[:, ot * 128:(ot + 1) * 128])
        m["w2"] = w2cols.astype(mm_np)

        m["par"] = par
        in_maps.append(m)
    return in_maps


def _assemble(results):
    """Per-core outputs -> tuple of 3 full [2,16,3,h,w,5] arrays."""
    parts = []
    for core in range(N_CORES):
        r = results[core]["r0"].reshape(NT, NO, NA, BS, HS0, HW[0])
        parts.append(r.transpose(3, 0, 2, 4, 5, 1))
    o0 = np.ascontiguousarray(np.concatenate(parts, axis=3))

    full1 = np.zeros((NTJ, NA, P1F), np.float64)
    for core in range(N_CORES):
        q1, ots1 = _l1_tasks(core)
        rc = results[core]["r1"]
        for t, ot in enumerate(ots1):
            full1[:, ot // 4, q1 * L1P:(q1 + 1) * L1P] += \
                rc[:, t * L1P:(t + 1) * L1P]
    o1 = full1.astype(np.float32).reshape(NT, NO, NA, BS, HW[1], HW[1])
    o1 = np.ascontiguousarray(o1.transpose(3, 0, 2, 4, 5, 1))

    full2 = np.zeros((NTJ, NA, P2F), np.float64)
    for core in range(N_CORES):
        h2, ots2 = _l2_tasks(core)
        rc = results[core]["r2"]
        for t, ot in enumerate(ots2):
            full2[:, ot // 4, h2 * L2P:(h2 + 1) * L2P] += \
                rc[:, t * L2P:(t + 1) * L2P]
    o2 = full2.astype(np.float32).reshape(NT, NO, NA, BS, HW[2], HW[2])
    o2 = np.ascontiguousarray(o2.transpose(3, 0, 2, 4, 5, 1))
    return (o0, o1, o2)


def _get_nc():
    if "nc" not in _STATE:
        _STATE["nc"] = _build()
    return _STATE["nc"]


def _run(inputs, **kw):
    nc = _get_nc()
    in_maps = _prep(inputs)
    res = run_bass_kernel_spmd(nc, in_maps, list(range(N_CORES)), **kw)
    return res


def kernel(**inputs):
    res = _run(inputs)
    return _assemble(res.results)
